# revision 1
# baseline (speedup 1.0000x reference)
"""EntropyBottleneck forward (q_mode='noise') as a Trainium2 Bass kernel.

Math
----
reference computes, per channel c with tiny per-channel params (W_k, b_k, f_k):

    y    = x + noise
    v    = y flattened per channel
    L(v) = chain of FactorizeCell: u <- softplus(W_k) @ u + b_k,
           then u <- u + tanh(f_k) * tanh(u)   (for k < last)
    lower = L(v - 0.5); upper = L(v + 0.5)
    s     = -sign(lower + upper)
    lik   = max(|sigmoid(s*upper) - sigmoid(s*lower)|, 1e-9)

When every gate f_k == 0 (true for this module's initialization), the chain is
per-channel *affine*: L(v) = M_c * v + D_c, with M_c > 0 (product of softplus
matrices) and D_c foldable on the host from the (C,3,3)-at-most params.
Then with h = M_c/2:

    lower = t - h,  upper = t + h,  where t = M_c * y + D_c
    lik   = |sigmoid(s*(t+h)) - sigmoid(s*(t-h))|
          = sigmoid(h - |t|) - sigmoid(-h - |t|)           (sign trick folded)
          = 0.5 * (tanh((t + h)/2) - tanh((t - h)/2))      (tanh identity,
                                                            sign-free: always >= 0)

The device kernel therefore does, per element:
    y   = x + noise                                  (vector engine)
    p   = tanh(M/2 * y + (D + h)/2)                  (scalar engine, fused affine)
    q   = tanh(M/2 * y + (D - h)/2)                  (scalar engine, fused affine)
    lik = max(0.5 * (p - q), 1e-9)                   (vector engine, fused)

Sharding: data-parallel over batch, one batch element per NeuronCore (8 cores).
Per-core tensor (192, 4096) is viewed as (384, 2048): row r holds half of
channel r//2, so each SBUF partition maps to exactly one channel and the
per-channel coefficients become per-partition scale/bias operands.
"""

import numpy as np

B, C, H, W = 8, 192, 64, 64
NCORES = 8
ROWS, COLS = 384, 2048  # (C, H*W) = (192, 4096) viewed as (384, 2048)
NT = ROWS // 128  # 3 row-tiles of 128 partitions

_CACHE: dict = {}


def _softplus64(x: np.ndarray) -> np.ndarray:
    x = x.astype(np.float64)
    return np.log1p(np.exp(-np.abs(x))) + np.maximum(x, 0.0)


def _fold_affine(ws, bs):
    """Compose the per-channel affine chain: L(v) = M*v + D. Returns (M, D) as (C,)."""
    M = np.ones((C, 1, 1), np.float64)
    D = np.zeros((C, 1, 1), np.float64)
    for Wk, bk in zip(ws, bs):
        spw = _softplus64(np.asarray(Wk))
        M = spw @ M
        D = spw @ D + np.asarray(bk, np.float64)
    return M[:, 0, 0], D[:, 0, 0]


def _numpy_fallback(x, noise, ws, bs, fs):
    """Exact replica of the reference chain for the general (gated) case."""
    x = np.asarray(x, np.float32)
    noise = np.asarray(noise, np.float32)
    y = x + noise
    v = y.transpose(1, 0, 2, 3).reshape(C, 1, -1).astype(np.float32)

    def logits(v):
        for i, (Wk, bk) in enumerate(zip(ws, bs)):
            spw = _softplus64(np.asarray(Wk)).astype(np.float32)
            v = np.einsum("coi,cin->con", spw, v) + np.asarray(bk, np.float32)
            if i < len(fs):
                v = v + np.tanh(np.asarray(fs[i], np.float32)) * np.tanh(v)
        return v

    lower = logits(v - 0.5)
    upper = logits(v + 0.5)
    sign = -np.sign(lower + upper)
    sig = lambda z: 1.0 / (1.0 + np.exp(-z, dtype=np.float32))
    lik = np.abs(sig(sign * upper) - sig(sign * lower))
    lik = np.maximum(lik, np.float32(1e-9))
    lik = lik.reshape(C, B, H, W).transpose(1, 0, 2, 3)
    return y, lik


def _build_program():
    import concourse.bacc as bacc
    import concourse.mybir as mybir
    import concourse.tile as tile

    f32 = mybir.dt.float32
    nc = bacc.Bacc("TRN2", target_bir_lowering=False, debug=False,
                   num_devices=NCORES)

    x_d = nc.dram_tensor("x", [ROWS, COLS], f32, kind="ExternalInput")
    n_d = nc.dram_tensor("noise", [ROWS, COLS], f32, kind="ExternalInput")
    sc_d = nc.dram_tensor("scl", [128, NT], f32, kind="ExternalInput")
    bp_d = nc.dram_tensor("bp", [128, NT], f32, kind="ExternalInput")
    bq_d = nc.dram_tensor("bq", [128, NT], f32, kind="ExternalInput")
    y_d = nc.dram_tensor("y", [ROWS, COLS], f32, kind="ExternalOutput")
    l_d = nc.dram_tensor("lik", [ROWS, COLS], f32, kind="ExternalOutput")

    Tanh = mybir.ActivationFunctionType.Tanh
    op_add = mybir.AluOpType.add
    op_sub = mybir.AluOpType.subtract
    op_mult = mybir.AluOpType.mult
    op_max = mybir.AluOpType.max

    with tile.TileContext(nc) as tc:
        with (
            tc.tile_pool(name="const", bufs=1) as cpool,
            tc.tile_pool(name="io", bufs=3) as iopool,
            tc.tile_pool(name="tmp", bufs=3) as tpool,
        ):
            sc = cpool.tile([128, NT], f32, tag="sc")
            nc.sync.dma_start(sc[:], sc_d[:])
            bp = cpool.tile([128, NT], f32, tag="bp")
            nc.sync.dma_start(bp[:], bp_d[:])
            bq = cpool.tile([128, NT], f32, tag="bq")
            nc.sync.dma_start(bq[:], bq_d[:])

            for t in range(NT):
                rows = slice(t * 128, (t + 1) * 128)
                xt = iopool.tile([128, COLS], f32, tag="xt")
                nc.sync.dma_start(xt[:], x_d[rows, :])
                nt = iopool.tile([128, COLS], f32, tag="nt")
                nc.sync.dma_start(nt[:], n_d[rows, :])

                yt = iopool.tile([128, COLS], f32, tag="yt")
                nc.vector.tensor_tensor(yt[:], xt[:], nt[:], op=op_add)
                nc.sync.dma_start(y_d[rows, :], yt[:])

                pt = tpool.tile([128, COLS], f32, tag="pt")
                nc.scalar.activation(pt[:], yt[:], Tanh,
                                     bias=bp[:, t:t + 1], scale=sc[:, t:t + 1])
                qt = tpool.tile([128, COLS], f32, tag="qt")
                nc.scalar.activation(qt[:], yt[:], Tanh,
                                     bias=bq[:, t:t + 1], scale=sc[:, t:t + 1])

                dt = tpool.tile([128, COLS], f32, tag="dt")
                nc.vector.tensor_tensor(dt[:], pt[:], qt[:], op=op_sub)
                lt = tpool.tile([128, COLS], f32, tag="lt")
                nc.vector.tensor_scalar(lt[:], dt[:], 0.5, 1e-9,
                                        op0=op_mult, op1=op_max)
                nc.sync.dma_start(l_d[rows, :], lt[:])

    nc.compile()
    return nc


def _get_program():
    if "nc" not in _CACHE:
        _CACHE["nc"] = _build_program()
    return _CACHE["nc"]


def kernel(x, noise, w0, b0, f0, w1, b1, f1, w2, b2, f2, w3, b3):
    from concourse.bass_utils import run_bass_kernel_spmd

    ws = [w0, w1, w2, w3]
    bs = [b0, b1, b2, b3]
    fs = [f0, f1, f2]

    if any(np.any(np.asarray(f) != 0.0) for f in fs):
        # Gated (non-affine) case: bit-accurate host fallback. Never taken for
        # this module's initialization (all gates are zero).
        return _numpy_fallback(x, noise, ws, bs, fs)

    M, D = _fold_affine(ws, bs)  # (C,) float64 each, M > 0
    ch = np.arange(ROWS) // 2  # channel id per folded row
    Mr, Dr = M[ch], D[ch]
    # p/q = tanh(M/2 * y + (D +- M/2)/2)
    scl = (Mr / 2).astype(np.float32).reshape(NT, 128).T.copy()
    bpv = (Dr / 2 + Mr / 4).astype(np.float32).reshape(NT, 128).T.copy()
    bqv = (Dr / 2 - Mr / 4).astype(np.float32).reshape(NT, 128).T.copy()

    x = np.ascontiguousarray(np.asarray(x, np.float32))
    noise = np.ascontiguousarray(np.asarray(noise, np.float32))

    nc = _get_program()
    in_maps = [
        {
            "x": x[b].reshape(ROWS, COLS),
            "noise": noise[b].reshape(ROWS, COLS),
            "scl": scl,
            "bp": bpv,
            "bq": bqv,
        }
        for b in range(NCORES)
    ]
    res = run_bass_kernel_spmd(nc, in_maps, list(range(NCORES))).results

    y = np.stack([res[b]["y"].reshape(C, H, W) for b in range(NCORES)])
    lik = np.stack([res[b]["lik"].reshape(C, H, W) for b in range(NCORES)])
    return y, lik


# revision 2
# speedup vs baseline: 1.1565x; 1.1565x over previous
"""EntropyBottleneck forward (q_mode='noise') as a Trainium2 Bass kernel.

Math
----
reference computes, per channel c with tiny per-channel params (W_k, b_k, f_k):

    y    = x + noise
    v    = y flattened per channel
    L(v) = chain of FactorizeCell: u <- softplus(W_k) @ u + b_k,
           then u <- u + tanh(f_k) * tanh(u)   (for k < last)
    lower = L(v - 0.5); upper = L(v + 0.5)
    s     = -sign(lower + upper)
    lik   = max(|sigmoid(s*upper) - sigmoid(s*lower)|, 1e-9)

When every gate f_k == 0 (true for this module's initialization), the chain is
per-channel *affine*: L(v) = M_c * v + D_c, with M_c > 0 (product of softplus
matrices) and D_c foldable on the host from the (C,3,3)-at-most params.
Then with h = M_c/2:

    lower = t - h,  upper = t + h,  where t = M_c * y + D_c
    lik   = |sigmoid(s*(t+h)) - sigmoid(s*(t-h))|
          = sigmoid(h - |t|) - sigmoid(-h - |t|)           (sign trick folded)
          = 0.5 * (tanh((t + h)/2) - tanh((t - h)/2))      (tanh identity,
                                                            sign-free: always >= 0)

The device kernel therefore does, per element:
    y   = x + noise                                  (vector engine)
    p   = tanh(M/2 * y + (D + h)/2)                  (scalar engine, fused affine)
    q   = tanh(M/2 * y + (D - h)/2)                  (scalar engine, fused affine)
    lik = max(0.5 * (p - q), 1e-9)                   (vector engine, fused)

Sharding: data-parallel over batch, one batch element per NeuronCore (8 cores).
Per-core tensor (192, 4096) is viewed as (384, 2048): row r holds half of
channel r//2, so each SBUF partition maps to exactly one channel and the
per-channel coefficients become per-partition scale/bias operands.
"""

import numpy as np

B, C, H, W = 8, 192, 64, 64
NCORES = 8
ROWS, COLS = 384, 2048  # (C, H*W) = (192, 4096) viewed as (384, 2048)
NT = ROWS // 128  # 3 row-tiles of 128 partitions

_CACHE: dict = {}


def _softplus64(x: np.ndarray) -> np.ndarray:
    x = x.astype(np.float64)
    return np.log1p(np.exp(-np.abs(x))) + np.maximum(x, 0.0)


def _fold_affine(ws, bs):
    """Compose the per-channel affine chain: L(v) = M*v + D. Returns (M, D) as (C,)."""
    M = np.ones((C, 1, 1), np.float64)
    D = np.zeros((C, 1, 1), np.float64)
    for Wk, bk in zip(ws, bs):
        spw = _softplus64(np.asarray(Wk))
        M = spw @ M
        D = spw @ D + np.asarray(bk, np.float64)
    return M[:, 0, 0], D[:, 0, 0]


def _numpy_fallback(x, noise, ws, bs, fs):
    """Exact replica of the reference chain for the general (gated) case."""
    x = np.asarray(x, np.float32)
    noise = np.asarray(noise, np.float32)
    y = x + noise
    v = y.transpose(1, 0, 2, 3).reshape(C, 1, -1).astype(np.float32)

    def logits(v):
        for i, (Wk, bk) in enumerate(zip(ws, bs)):
            spw = _softplus64(np.asarray(Wk)).astype(np.float32)
            v = np.einsum("coi,cin->con", spw, v) + np.asarray(bk, np.float32)
            if i < len(fs):
                v = v + np.tanh(np.asarray(fs[i], np.float32)) * np.tanh(v)
        return v

    lower = logits(v - 0.5)
    upper = logits(v + 0.5)
    sign = -np.sign(lower + upper)
    sig = lambda z: 1.0 / (1.0 + np.exp(-z, dtype=np.float32))
    lik = np.abs(sig(sign * upper) - sig(sign * lower))
    lik = np.maximum(lik, np.float32(1e-9))
    lik = lik.reshape(C, B, H, W).transpose(1, 0, 2, 3)
    return y, lik


def _build_program():
    import concourse.bacc as bacc
    import concourse.mybir as mybir
    import concourse.tile as tile

    f32 = mybir.dt.float32
    nc = bacc.Bacc("TRN2", target_bir_lowering=False, debug=False,
                   num_devices=NCORES)

    x_d = nc.dram_tensor("x", [ROWS, COLS], f32, kind="ExternalInput")
    n_d = nc.dram_tensor("noise", [ROWS, COLS], f32, kind="ExternalInput")
    sc_d = nc.dram_tensor("scl", [128, NT], f32, kind="ExternalInput")
    bp_d = nc.dram_tensor("bp", [128, NT], f32, kind="ExternalInput")
    bq_d = nc.dram_tensor("bq", [128, NT], f32, kind="ExternalInput")
    y_d = nc.dram_tensor("y", [ROWS, COLS], f32, kind="ExternalOutput")
    l_d = nc.dram_tensor("lik", [ROWS, COLS], f32, kind="ExternalOutput")

    Tanh = mybir.ActivationFunctionType.Tanh
    op_add = mybir.AluOpType.add
    op_sub = mybir.AluOpType.subtract
    op_mult = mybir.AluOpType.mult
    op_max = mybir.AluOpType.max

    with tile.TileContext(nc) as tc:
        with (
            tc.tile_pool(name="const", bufs=1) as cpool,
            tc.tile_pool(name="io", bufs=1) as iopool,
            tc.tile_pool(name="tmp", bufs=2) as tpool,
        ):
            sc = cpool.tile([128, NT], f32, tag="sc")
            nc.sync.dma_start(sc[:], sc_d[:])
            bp = cpool.tile([128, NT], f32, tag="bp")
            nc.sync.dma_start(bp[:], bp_d[:])
            bq = cpool.tile([128, NT], f32, tag="bq")
            nc.sync.dma_start(bq[:], bq_d[:])

            # All loads issue first on the sync (HWDGE) ring, in tile order;
            # stores go out on the gpsimd (SWDGE) ring so they never queue
            # behind loads in the HWDGE FIFO.
            xts, nts = [], []
            for t in range(NT):
                rows = slice(t * 128, (t + 1) * 128)
                xt = iopool.tile([128, COLS], f32, tag=f"xt{t}")
                nc.sync.dma_start(xt[:], x_d[rows, :])
                nt = iopool.tile([128, COLS], f32, tag=f"nt{t}")
                nc.sync.dma_start(nt[:], n_d[rows, :])
                xts.append(xt)
                nts.append(nt)

            for t in range(NT):
                rows = slice(t * 128, (t + 1) * 128)
                yt = iopool.tile([128, COLS], f32, tag=f"yt{t}")
                nc.vector.tensor_tensor(yt[:], xts[t][:], nts[t][:], op=op_add)
                nc.gpsimd.dma_start(y_d[rows, :], yt[:])

                pt = tpool.tile([128, COLS], f32, tag="pt")
                nc.scalar.activation(pt[:], yt[:], Tanh,
                                     bias=bp[:, t:t + 1], scale=sc[:, t:t + 1])
                qt = tpool.tile([128, COLS], f32, tag="qt")
                nc.scalar.activation(qt[:], yt[:], Tanh,
                                     bias=bq[:, t:t + 1], scale=sc[:, t:t + 1])

                nc.vector.tensor_tensor(pt[:], pt[:], qt[:], op=op_sub)
                nc.vector.tensor_scalar(pt[:], pt[:], 0.5, 1e-9,
                                        op0=op_mult, op1=op_max)
                nc.gpsimd.dma_start(l_d[rows, :], pt[:])

    nc.compile()
    return nc


def _get_program():
    if "nc" not in _CACHE:
        _CACHE["nc"] = _build_program()
    return _CACHE["nc"]


def kernel(x, noise, w0, b0, f0, w1, b1, f1, w2, b2, f2, w3, b3):
    from concourse.bass_utils import run_bass_kernel_spmd

    ws = [w0, w1, w2, w3]
    bs = [b0, b1, b2, b3]
    fs = [f0, f1, f2]

    if any(np.any(np.asarray(f) != 0.0) for f in fs):
        # Gated (non-affine) case: bit-accurate host fallback. Never taken for
        # this module's initialization (all gates are zero).
        return _numpy_fallback(x, noise, ws, bs, fs)

    M, D = _fold_affine(ws, bs)  # (C,) float64 each, M > 0
    ch = np.arange(ROWS) // 2  # channel id per folded row
    Mr, Dr = M[ch], D[ch]
    # p/q = tanh(M/2 * y + (D +- M/2)/2)
    scl = (Mr / 2).astype(np.float32).reshape(NT, 128).T.copy()
    bpv = (Dr / 2 + Mr / 4).astype(np.float32).reshape(NT, 128).T.copy()
    bqv = (Dr / 2 - Mr / 4).astype(np.float32).reshape(NT, 128).T.copy()

    x = np.ascontiguousarray(np.asarray(x, np.float32))
    noise = np.ascontiguousarray(np.asarray(noise, np.float32))

    nc = _get_program()
    in_maps = [
        {
            "x": x[b].reshape(ROWS, COLS),
            "noise": noise[b].reshape(ROWS, COLS),
            "scl": scl,
            "bp": bpv,
            "bq": bqv,
        }
        for b in range(NCORES)
    ]
    res = run_bass_kernel_spmd(nc, in_maps, list(range(NCORES))).results

    y = np.stack([res[b]["y"].reshape(C, H, W) for b in range(NCORES)])
    lik = np.stack([res[b]["lik"].reshape(C, H, W) for b in range(NCORES)])
    return y, lik
